# revision 22
# baseline (speedup 1.0000x reference)
"""Sparse (sigmoid) attention block on 8 TRN2 NeuronCores.

Sharding: core c = (batch b=c//2, head-half hh=c%2).  Each core computes the
QKV projection + RoPE + causal sigmoid-attention for 6 of the 12 heads over
the full 2048-row sequence of its batch (graphs are identical across cores;
only the weight/input shards differ).  A 2-rank AllGather inside each core
pair exchanges the per-head attention outputs so both cores see the full
hidden dim for LayerNorm; the epilogue output projection is column-split
across the pair.  All heavy compute in bf16 with f32 PSUM accumulation.

Layouts: projections produce row-major [rows, cols] tiles; RoPE is applied
in row layout (free-axis half-rotation via a sign-folded sin table); PE
transposes produce head-pair-stacked Q^T/K^T [128, pair, seq]; attention
output, LayerNorm, gating and the output projection all live in transposed
[hidden, rows] layout so no further transposes are needed; the host
transposes the final [384, 2048] per-core outputs back.
"""

import numpy as np
import ml_dtypes

import concourse.bass as bass
import concourse.bacc as bacc
import concourse.mybir as mybir
import concourse.tile as tile
from concourse import bass_utils

BF16 = mybir.dt.bfloat16
F32 = mybir.dt.float32
AF = mybir.ActivationFunctionType

S = 2048          # sequence length
HID = 768         # hidden
NHEADS = 12       # total heads
D = 64            # head dim
NH = 6            # heads per core
NPAIR = 3         # head pairs per core
KC = 16           # key chunks of 128
NRB = 4           # row blocks of 512
RB = 512
CT = 6            # hidden c-tiles of 128
LN_EPS = 1e-8
N_CORES = 8


def _rope_tables():
    inv_freq = 1.0 / (10000.0 ** (np.arange(0, D, 2, dtype=np.float64) / D))
    t = np.arange(S, dtype=np.float64)
    freqs = np.outer(t, inv_freq)                      # [S, 32]
    emb = np.concatenate([freqs, freqs], axis=-1)      # [S, 64]
    return np.cos(emb).astype(np.float32), np.sin(emb).astype(np.float32)


def build_nc(ndev, pairs):
    """Emit the per-core Bass/Tile graph (identical for every core)."""
    nc = bacc.Bacc("TRN2", target_bir_lowering=False, debug=False,
                   num_devices=ndev)

    def din(name, shape, dt):
        return nc.dram_tensor(name, shape, dt, kind="ExternalInput").ap()

    xT = din("xT", [HID, S], BF16)
    w_qkv = din("w_qkv", [HID, 3 * NH * D], BF16)      # [768, 1152] Q|K|V
    w_u = din("w_u", [HID, HID], BF16)
    w_out = din("w_out", [HID, 384], BF16)             # gamma-folded half
    cosr = din("cosr", [S, NH * D], BF16)
    sinr = din("sinr", [S, NH * D], BF16)              # sign-folded sin
    maskb = din("maskb", [128, 896], BF16)
    ident = din("ident", [128, 128], BF16)
    ones_k = din("ones_k", [128, 1], BF16)
    ones_m = din("ones_m", [1, 128], BF16)
    residT = din("residT", [384, S], F32)              # x^T half + b_out
    out = nc.dram_tensor("out", [384, S], F32, kind="ExternalOutput").ap()

    xT_r = xT.rearrange("(k p) s -> p k s", p=128)         # [128, 6, S]
    wqkv_r = w_qkv.rearrange("(k p) c -> p k c", p=128)    # [128, 6, 1152]
    wu_r = w_u.rearrange("(k p) c -> p k c", p=128)
    wout_r = w_out.rearrange("(k p) c -> p k c", p=128)    # [128, 6, 384]
    residT_r = residT.rearrange("(c p) s -> p c s", p=128)  # [128, 3, S]
    out_r = out.rearrange("(c p) s -> p c s", p=128)

    with tile.TileContext(nc) as tc:
        _emit(nc, tc, pairs, xT_r, wqkv_r, wu_r, wout_r, cosr, sinr,
              maskb, ident, ones_k, ones_m, residT_r, out_r)
    nc.compile()
    return nc


def _emit(nc, tc, pairs, xT_r, wqkv_r, wu_r, wout_r, cosr, sinr,
          maskb, ident, ones_k, ones_m, residT_r, out_r):
    from contextlib import ExitStack
    es = ExitStack()
    with es:
        # ---- resident SBUF tensors -----------------------------------
        res = es.enter_context(tc.tile_pool(name="resident", bufs=1))
        xT_sb = res.tile([128, 6, S], BF16, tag="xT")
        wqkv_sb = res.tile([128, 6, 3 * NH * D], BF16, tag="wqkv")
        wu_sb = res.tile([128, 6, HID], BF16, tag="wu")
        wout_sb = res.tile([128, 6, 384], BF16, tag="wout")
        maskb_sb = res.tile([128, 896], BF16, tag="maskb")
        ident_sb = res.tile([128, 128], BF16, tag="ident")
        ones_k_sb = res.tile([128, 1], BF16, tag="onesk")
        ones_m_sb = res.tile([1, 128], BF16, tag="onesm")
        # per-row-block slabs so attention on early query blocks can start
        # while projection of later rows is still running
        qt_sb = [res.tile([128, NPAIR, RB], BF16, tag=f"qt{i}", name=f"qt{i}")
                 for i in range(NRB)]                          # Q^T slabs
        kt_sb = [res.tile([128, NPAIR, RB], BF16, tag=f"kt{i}", name=f"kt{i}")
                 for i in range(NRB)]                          # K^T slabs
        v_sb = [res.tile([128, 4, NH, D], BF16, tag=f"v{i}", name=f"v{i}")
                for i in range(NRB)]                           # V slabs
        ao_sb = [res.tile([128, NPAIR, RB], BF16, tag=f"ao{i}", name=f"ao{i}")
                 for i in range(NRB)]                          # attn out^T
        ut_sb = res.tile([128, CT, S], BF16, tag="ut")         # U^T slab

        for k in range(6):
            nc.sync.dma_start(out=xT_sb[:, k, :], in_=xT_r[:, k, :])
            nc.sync.dma_start(out=wqkv_sb[:, k, :], in_=wqkv_r[:, k, :])
            nc.sync.dma_start(out=wu_sb[:, k, :], in_=wu_r[:, k, :])
            nc.sync.dma_start(out=wout_sb[:, k, :], in_=wout_r[:, k, :])
        nc.sync.dma_start(out=maskb_sb[:], in_=maskb[:])
        nc.sync.dma_start(out=ident_sb[:], in_=ident[:])
        nc.sync.dma_start(out=ones_k_sb[:], in_=ones_k[:])
        nc.sync.dma_start(out=ones_m_sb[:], in_=ones_m[:])

        # ---- phase 1: QKV projection + RoPE + transposes -------------
        # PSUM pools for phases 1 and 2 coexist within the 8 banks so the
        # scheduler can overlap projection with early attention blocks.
        with tc.tile_pool(name="p1psum", bufs=1, space="PSUM") as pp, \
             tc.tile_pool(name="p1sb", bufs=3) as sb:
            for rt in range(KC):
                # Q at [0:384] (bank0) and K at [512:896] (bank1) of one
                # 2-bank tile; V time-shares the same slot tag so phase-1
                # PSUM stays at 2 banks and phase-2 pools fit alongside.
                pqk = pp.tile([128, 1024], F32, tag="pqk")
                for k in range(6):
                    lhs = xT_sb[:, k, rt * 128:(rt + 1) * 128]
                    st, sp = (k == 0), (k == 5)
                    nc.tensor.matmul(pqk[:, 0:384], lhs, wqkv_sb[:, k, 0:384],
                                     start=st, stop=sp)
                    nc.tensor.matmul(pqk[:, 512:896], lhs,
                                     wqkv_sb[:, k, 384:768],
                                     start=st, stop=sp)
                rb, rt4 = rt // 4, rt % 4
                r0, r1 = rt * 128, (rt + 1) * 128
                c0, c1 = rt4 * 128, (rt4 + 1) * 128
                # Q/K to SBUF bf16 then RoPE in row layout
                q_t = sb.tile([128, NH, D], BF16, tag="qrow")
                k_t = sb.tile([128, NH, D], BF16, tag="krow")
                nc.scalar.copy(q_t[:], pqk[:, 0:384])
                nc.scalar.copy(k_t[:], pqk[:, 512:896])
                pv = pp.tile([128, 1024], F32, tag="pqk", name="pv")
                for k in range(6):
                    nc.tensor.matmul(pv[:, 0:384],
                                     xT_sb[:, k, rt * 128:(rt + 1) * 128],
                                     wqkv_sb[:, k, 768:1152],
                                     start=(k == 0), stop=(k == 5))
                # V straight to slab (bf16)
                nc.scalar.copy(v_sb[rb][:, rt4, :, :], pv[:, 0:384])
                cos_t = sb.tile([128, NH, D], BF16, tag="cos")
                sin_t = sb.tile([128, NH, D], BF16, tag="sin")
                nc.sync.dma_start(out=cos_t[:], in_=cosr[r0:r1, :])
                nc.sync.dma_start(out=sin_t[:], in_=sinr[r0:r1, :])
                for name, src in (("q", q_t), ("k", k_t)):
                    t1 = sb.tile([128, NH, D], BF16, tag="rope_t1")
                    t2 = sb.tile([128, NH, D], BF16, tag="rope_t2")
                    rr = sb.tile([128, NH, D], BF16, tag="rope_r")
                    nc.vector.tensor_mul(t1[:, :, 0:32], src[:, :, 32:64],
                                         sin_t[:, :, 0:32])
                    nc.vector.tensor_mul(t1[:, :, 32:64], src[:, :, 0:32],
                                         sin_t[:, :, 32:64])
                    nc.vector.tensor_mul(t2[:], src[:], cos_t[:])
                    nc.vector.tensor_add(rr[:], t1[:], t2[:])
                    dst = qt_sb if name == "q" else kt_sb
                    for p in range(NPAIR):
                        tp = pp.tile([128, 128], BF16, tag="tp", name="tp")
                        nc.tensor.transpose(tp[:], rr[:, 2 * p:2 * p + 2, :],
                                            ident_sb[:])
                        nc.scalar.copy(dst[rb][:, p, c0:c1], tp[:])

            # phase 1b: U^T projection (W stationary); silu applied here
            # (it does not depend on the AllGather) so the epilogue only
            # multiplies.
            for ct in range(CT):
                for rb in range(NRB):
                    pu = pp.tile([128, RB], F32, tag="pqk", name="pu")
                    for k in range(6):
                        nc.tensor.matmul(
                            pu[:], wu_sb[:, k, ct * 128:(ct + 1) * 128],
                            xT_sb[:, k, rb * RB:(rb + 1) * RB],
                            start=(k == 0), stop=(k == 5))
                    usig = sb.tile([128, RB], BF16, tag="usig")
                    nc.scalar.activation(usig[:], pu[:], AF.Sigmoid)
                    nc.vector.tensor_mul(
                        ut_sb[:, ct, rb * RB:(rb + 1) * RB], usig[:], pu[:])

            # ---- phase 2: attention + chunked AllGather --------------
            dram = es.enter_context(tc.tile_pool(name="agdram", bufs=4,
                                                 space="DRAM"))
            ag_outs = []
            p2 = ExitStack()
            scp = p2.enter_context(
                tc.tile_pool(name="p2sc", bufs=2, space="PSUM"))
            avp = p2.enter_context(
                tc.tile_pool(name="p2av", bufs=1, space="PSUM"))
            atp = p2.enter_context(tc.tile_pool(name="p2sb", bufs=17))
            for qb in range(NRB):
                nkc = 4 * qb + 4
                q0, q1 = qb * RB, (qb + 1) * RB
                for p in range(NPAIR):
                    av = avp.tile([128, RB], F32, tag="av")
                    # all scores matmuls of the head pair back-to-back so
                    # the in-order PE never stalls on a sigmoid (keeps the
                    # HAM clock gate open); sigmoids pipeline on ScalarE.
                    # one [128,1024] PSUM tile per key chunk holds BOTH
                    # heads' scores: the two matmuls target PE row groups
                    # 0-63 / 64-127 so each LDWEIGHTS overlaps the other
                    # head's matmul, and one big sigmoid serves both.
                    ats = {}
                    for kc in range(nkc):
                        sc = scp.tile([128, 1024], F32, tag="sc")
                        at = atp.tile([128, 1024], BF16, tag="at")
                        kslc = kt_sb[kc // 4]
                        for h01 in range(2):
                            b0 = 64 * h01
                            nc.tensor.matmul(
                                sc[:, h01 * RB:(h01 + 1) * RB],
                                kslc[b0:b0 + 64, p,
                                     (kc % 4) * 128:(kc % 4 + 1) * 128],
                                qt_sb[qb][b0:b0 + 64, p, :],
                                start=True, stop=True)
                        nc.scalar.activation(at[:], sc[:], AF.Sigmoid,
                                             scale=0.125)
                        t = kc - 4 * qb
                        if t >= 0:
                            for h01 in range(2):
                                nc.vector.tensor_mul(
                                    at[:, h01 * RB:(h01 + 1) * RB],
                                    at[:, h01 * RB:(h01 + 1) * RB],
                                    maskb_sb[:, 384 - 128 * t:896 - 128 * t])
                        ats[kc] = at
                    # A@V: the h0/h1 matmuls write disjoint PSUM partition
                    # halves (PE column groups 0-63 / 64-127) back-to-back,
                    # which the PE executes concurrently.
                    for kc in range(nkc):
                        for h01 in range(2):
                            b0 = 64 * h01
                            nc.tensor.matmul(
                                av[b0:b0 + 64, :],
                                v_sb[kc // 4][:, kc % 4, 2 * p + h01, :],
                                ats[kc][:, h01 * RB:(h01 + 1) * RB],
                                start=(kc == 0), stop=(kc == nkc - 1),
                                skip_group_check=True)
                    nc.vector.tensor_copy(ao_sb[qb][:, p, :], av[:])
                ag_in = dram.tile([NPAIR, 128, RB], BF16, tag="agin")
                ag_out = dram.tile([2, NPAIR, 128, RB], BF16, tag="agout")
                nc.gpsimd.dma_start(out=ag_in.rearrange("p i j -> i p j"),
                                    in_=ao_sb[qb][:])
                nc.gpsimd.collective_compute(
                    "AllGather", mybir.AluOpType.bypass,
                    replica_groups=pairs,
                    ins=[ag_in.opt()], outs=[ag_out.opt()])
                ag_outs.append(ag_out)
            p2.close()

        # ---- phase 3: LayerNorm + gate + out proj + residual ---------
        with tc.tile_pool(name="p3st", bufs=2, space="PSUM") as stp, \
             tc.tile_pool(name="p3bc", bufs=1, space="PSUM") as bcp, \
             tc.tile_pool(name="p3o", bufs=2, space="PSUM") as op, \
             tc.tile_pool(name="p3sb", bufs=2) as sb, \
             tc.tile_pool(name="p3small", bufs=2) as ssb:
            for rb in range(NRB):
                aot = sb.tile([128, 2, NPAIR, RB], BF16, tag="aot")
                nc.sync.dma_start(
                    out=aot[:],
                    in_=ag_outs[rb].rearrange("r p i j -> i r p j"))
                aotf = aot.rearrange("i r p j -> i (r p) j")   # [128, 6, RB]
                ssum = stp.tile([1, RB], F32, tag="ssum")
                qsum = stp.tile([1, RB], F32, tag="qsum")
                for ct in range(CT):
                    sq = sb.tile([128, RB], BF16, tag="sq")
                    nc.vector.tensor_mul(sq[:], aotf[:, ct, :], aotf[:, ct, :])
                    nc.tensor.matmul(ssum[:], ones_k_sb[:], aotf[:, ct, :],
                                     start=(ct == 0), stop=(ct == 5))
                    nc.tensor.matmul(qsum[:], ones_k_sb[:], sq[:],
                                     start=(ct == 0), stop=(ct == 5))
                mu = ssb.tile([1, RB], F32, tag="mu")
                musq = ssb.tile([1, RB], F32, tag="musq")
                var = ssb.tile([1, RB], F32, tag="var")
                std = ssb.tile([1, RB], F32, tag="musq", name="std")
                rstd = ssb.tile([1, RB], F32, tag="var", name="rstd")
                mu_b = ssb.tile([1, RB], BF16, tag="mub")
                rstd_b = ssb.tile([1, RB], BF16, tag="rstdb")
                nc.vector.tensor_scalar_mul(mu[:], ssum[:], 1.0 / HID)
                nc.vector.tensor_mul(musq[:], mu[:], mu[:])
                nc.vector.scalar_tensor_tensor(
                    var[:], qsum[:], 1.0 / HID, musq[:],
                    op0=mybir.AluOpType.mult, op1=mybir.AluOpType.subtract)
                eps_t = ssb.tile([1, 1], F32, tag="eps")
                nc.gpsimd.memset(eps_t[:], LN_EPS)
                nc.scalar.activation(std[:], var[:], AF.Sqrt, bias=eps_t[:])
                nc.vector.reciprocal_approx_fast(rstd[:], std[:])
                nc.vector.tensor_copy(mu_b[:], mu[:])
                nc.vector.tensor_copy(rstd_b[:], rstd[:])
                mu128 = bcp.tile([128, RB], F32, tag="mu128")
                rstd128 = bcp.tile([128, RB], F32, tag="rstd128")
                nc.tensor.matmul(mu128[:], ones_m_sb[:], mu_b[:],
                                 start=True, stop=True)
                nc.tensor.matmul(rstd128[:], ones_m_sb[:], rstd_b[:],
                                 start=True, stop=True)
                # stage broadcast stats to SBUF bf16 so the DVE apply chain
                # runs in its 2x bf16 mode (PSUM operands force 1x)
                mu_s = sb.tile([128, RB], BF16, tag="mus")
                rs_s = sb.tile([128, RB], BF16, tag="rss")
                nc.scalar.copy(mu_s[:], mu128[:])
                nc.scalar.copy(rs_s[:], rstd128[:])
                gated = sb.tile([128, CT, RB], BF16, tag="gated", bufs=1)
                for ct in range(CT):
                    d1 = sb.tile([128, RB], BF16, tag="d1")
                    d2 = sb.tile([128, RB], BF16, tag="d2")
                    nc.vector.tensor_sub(d1[:], aotf[:, ct, :], mu_s[:])
                    nc.vector.tensor_mul(d2[:], d1[:], rs_s[:])
                    nc.vector.tensor_mul(gated[:, ct, :], d2[:],
                                         ut_sb[:, ct, rb * RB:(rb + 1) * RB])
                for ctp in range(NPAIR):
                    po = op.tile([128, RB], F32, tag="po")
                    for ct in range(CT):
                        nc.tensor.matmul(
                            po[:], wout_sb[:, ct, ctp * 128:(ctp + 1) * 128],
                            gated[:, ct, :], start=(ct == 0), stop=(ct == 5))
                    rt_t = sb.tile([128, RB], F32, tag="resid")
                    nc.sync.dma_start(
                        out=rt_t[:],
                        in_=residT_r[:, ctp, rb * RB:(rb + 1) * RB])
                    o_t = sb.tile([128, RB], F32, tag="osb")
                    nc.vector.tensor_add(o_t[:], po[:], rt_t[:])
                    nc.gpsimd.dma_start(
                        out=out_r[:, ctp, rb * RB:(rb + 1) * RB], in_=o_t[:])


# ---------------------------------------------------------------------------
# host side
# ---------------------------------------------------------------------------

def prep_inputs(x, attn_mask, W_proj, b_proj, ln_gamma, ln_beta, W_out, b_out):
    x = np.asarray(x, dtype=np.float32)
    W_proj = np.asarray(W_proj, dtype=np.float32)
    b_proj = np.asarray(b_proj, dtype=np.float32)
    ln_gamma = np.asarray(ln_gamma, dtype=np.float32)
    ln_beta = np.asarray(ln_beta, dtype=np.float32)
    W_out = np.asarray(W_out, dtype=np.float32)
    b_out = np.asarray(b_out, dtype=np.float32)

    tril = np.tril(np.ones((S, S), dtype=bool))
    am = np.asarray(attn_mask)
    if not all(np.array_equal(am[b], tril) for b in range(am.shape[0])):
        raise ValueError("kernel specialized for causal attn_mask")
    if np.any(b_proj != 0) or np.any(ln_beta != 0):
        raise ValueError("kernel specialized for zero b_proj / ln_beta")

    bf = ml_dtypes.bfloat16
    cos, sin = _rope_tables()
    sinN = sin.copy()
    sinN[:, 0:32] = -sinN[:, 0:32]
    cosr = np.tile(cos, (1, NH)).astype(bf)
    sinr = np.tile(sinN, (1, NH)).astype(bf)

    iw = np.arange(896)[None, :]
    ii = np.arange(128)[:, None]
    maskb = (iw >= ii + 384).astype(bf)
    ident = np.eye(128, dtype=bf)
    ones_k = np.ones((128, 1), dtype=bf)
    ones_m = np.ones((1, 128), dtype=bf)

    Wg = (ln_gamma[:, None] * W_out).astype(np.float32)   # gamma folded
    U_c, V_c, Q_c, K_c = 0, HID, 2 * HID, 3 * HID

    in_maps = []
    for c in range(N_CORES):
        b, hh = c // 2, c % 2
        heads = range(NH * hh, NH * hh + NH)
        qcols = np.concatenate(
            [np.arange(Q_c + h * D, Q_c + (h + 1) * D) for h in heads])
        kcols = qcols - Q_c + K_c
        vcols = qcols - Q_c + V_c
        w_qkv = np.concatenate(
            [W_proj[:, qcols], W_proj[:, kcols], W_proj[:, vcols]],
            axis=1).astype(bf)
        w_u = W_proj[:, U_c:U_c + HID].astype(bf)
        w_out_half = Wg[:, hh * 384:(hh + 1) * 384].astype(bf)
        xTb = x[b].T                                       # [768, 2048]
        residT = (xTb[hh * 384:(hh + 1) * 384, :]
                  + b_out[hh * 384:(hh + 1) * 384, None]).astype(np.float32)
        in_maps.append(dict(
            xT=np.ascontiguousarray(xTb).astype(bf),
            w_qkv=np.ascontiguousarray(w_qkv),
            w_u=np.ascontiguousarray(w_u),
            w_out=np.ascontiguousarray(w_out_half),
            cosr=cosr, sinr=sinr, maskb=maskb, ident=ident,
            ones_k=ones_k, ones_m=ones_m,
            residT=np.ascontiguousarray(residT),
        ))
    return in_maps


def assemble(results, B=4):
    full = np.empty((B, S, HID), dtype=np.float32)
    for c in range(N_CORES):
        b, hh = c // 2, c % 2
        full[b, :, hh * 384:(hh + 1) * 384] = results[c]["out"].T
    return full


_NC_CACHE = {}


def get_nc(ndev=N_CORES):
    if ndev not in _NC_CACHE:
        pairs = [[i, i + 1] for i in range(0, ndev, 2)]
        _NC_CACHE[ndev] = build_nc(ndev, pairs)
    return _NC_CACHE[ndev]


def kernel(**inputs):
    in_maps = prep_inputs(**inputs)
    nc = get_nc(N_CORES)
    res = bass_utils.run_bass_kernel_spmd(
        nc, in_maps, core_ids=list(range(N_CORES)))
    return assemble(res.results)


# revision 23
# speedup vs baseline: 1.0353x; 1.0353x over previous
"""Sparse (sigmoid) attention block on 8 TRN2 NeuronCores.

Sharding: core c = (batch b=c//2, head-half hh=c%2).  Each core computes the
QKV projection + RoPE + causal sigmoid-attention for 6 of the 12 heads over
the full 2048-row sequence of its batch (graphs are identical across cores;
only the weight/input shards differ).  A 2-rank AllGather inside each core
pair exchanges the per-head attention outputs so both cores see the full
hidden dim for LayerNorm; the epilogue output projection is column-split
across the pair.  All heavy compute in bf16 with f32 PSUM accumulation.

Layouts: projections produce row-major [rows, cols] tiles; RoPE is applied
in row layout (free-axis half-rotation via a sign-folded sin table); PE
transposes produce head-pair-stacked Q^T/K^T [128, pair, seq]; attention
output, LayerNorm, gating and the output projection all live in transposed
[hidden, rows] layout so no further transposes are needed; the host
transposes the final [384, 2048] per-core outputs back.
"""

import numpy as np
import ml_dtypes

import concourse.bass as bass
import concourse.bacc as bacc
import concourse.mybir as mybir
import concourse.tile as tile
from concourse import bass_utils

BF16 = mybir.dt.bfloat16
F32 = mybir.dt.float32
AF = mybir.ActivationFunctionType

S = 2048          # sequence length
HID = 768         # hidden
NHEADS = 12       # total heads
D = 64            # head dim
NH = 6            # heads per core
NPAIR = 3         # head pairs per core
KC = 16           # key chunks of 128
NRB = 4           # row blocks of 512
RB = 512
CT = 6            # hidden c-tiles of 128
LN_EPS = 1e-8
N_CORES = 8


def _rope_tables():
    inv_freq = 1.0 / (10000.0 ** (np.arange(0, D, 2, dtype=np.float64) / D))
    t = np.arange(S, dtype=np.float64)
    freqs = np.outer(t, inv_freq)                      # [S, 32]
    emb = np.concatenate([freqs, freqs], axis=-1)      # [S, 64]
    return np.cos(emb).astype(np.float32), np.sin(emb).astype(np.float32)


def build_nc(ndev, pairs):
    """Emit the per-core Bass/Tile graph (identical for every core)."""
    nc = bacc.Bacc("TRN2", target_bir_lowering=False, debug=False,
                   num_devices=ndev)

    def din(name, shape, dt):
        return nc.dram_tensor(name, shape, dt, kind="ExternalInput").ap()

    xT = din("xT", [HID, S], BF16)
    w_qkv = din("w_qkv", [HID, 3 * NH * D], BF16)      # [768, 1152] Q|K|V
    w_u = din("w_u", [HID, HID], BF16)
    w_out = din("w_out", [HID, 384], BF16)             # gamma-folded half
    cosr = din("cosr", [S, NH * D], BF16)
    sinr = din("sinr", [S, NH * D], BF16)              # sign-folded sin
    maskb = din("maskb", [128, 896], BF16)
    ident = din("ident", [128, 128], BF16)
    ones_k = din("ones_k", [128, 1], BF16)
    ones_m = din("ones_m", [1, 128], BF16)
    residT = din("residT", [384, S], F32)              # x^T half + b_out
    out = nc.dram_tensor("out", [384, S], F32, kind="ExternalOutput").ap()

    xT_r = xT.rearrange("(k p) s -> p k s", p=128)         # [128, 6, S]
    wqkv_r = w_qkv.rearrange("(k p) c -> p k c", p=128)    # [128, 6, 1152]
    wu_r = w_u.rearrange("(k p) c -> p k c", p=128)
    wout_r = w_out.rearrange("(k p) c -> p k c", p=128)    # [128, 6, 384]
    residT_r = residT.rearrange("(c p) s -> p c s", p=128)  # [128, 3, S]
    out_r = out.rearrange("(c p) s -> p c s", p=128)

    with tile.TileContext(nc) as tc:
        _emit(nc, tc, pairs, xT_r, wqkv_r, wu_r, wout_r, cosr, sinr,
              maskb, ident, ones_k, ones_m, residT_r, out_r)
    nc.compile()
    return nc


def _emit(nc, tc, pairs, xT_r, wqkv_r, wu_r, wout_r, cosr, sinr,
          maskb, ident, ones_k, ones_m, residT_r, out_r):
    from contextlib import ExitStack
    es = ExitStack()
    with es:
        # ---- resident SBUF tensors -----------------------------------
        res = es.enter_context(tc.tile_pool(name="resident", bufs=1))
        xT_sb = res.tile([128, 6, S], BF16, tag="xT")
        wqkv_sb = res.tile([128, 6, 3 * NH * D], BF16, tag="wqkv")
        wu_sb = res.tile([128, 6, HID], BF16, tag="wu")
        wout_sb = res.tile([128, 6, 384], BF16, tag="wout")
        maskb_sb = res.tile([128, 896], BF16, tag="maskb")
        ident_sb = res.tile([128, 128], BF16, tag="ident")
        ones_k_sb = res.tile([128, 1], BF16, tag="onesk")
        ones_m_sb = res.tile([1, 128], BF16, tag="onesm")
        # per-row-block slabs so attention on early query blocks can start
        # while projection of later rows is still running
        qt_sb = [res.tile([128, NPAIR, RB], BF16, tag=f"qt{i}", name=f"qt{i}")
                 for i in range(NRB)]                          # Q^T slabs
        kt_sb = [res.tile([128, NPAIR, RB], BF16, tag=f"kt{i}", name=f"kt{i}")
                 for i in range(NRB)]                          # K^T slabs
        v_sb = [res.tile([128, 4, NH, D], BF16, tag=f"v{i}", name=f"v{i}")
                for i in range(NRB)]                           # V slabs
        ao_sb = [res.tile([128, NPAIR, RB], BF16, tag=f"ao{i}", name=f"ao{i}")
                 for i in range(NRB)]                          # attn out^T
        ut_sb = res.tile([128, CT, S], BF16, tag="ut")         # U^T slab

        # projection-critical loads first so the first matmuls start early
        for k in range(6):
            nc.sync.dma_start(out=xT_sb[:, k, :], in_=xT_r[:, k, :])
            nc.sync.dma_start(out=wqkv_sb[:, k, :], in_=wqkv_r[:, k, :])
        nc.sync.dma_start(out=ident_sb[:], in_=ident[:])
        nc.sync.dma_start(out=maskb_sb[:], in_=maskb[:])
        for k in range(6):
            nc.sync.dma_start(out=wu_sb[:, k, :], in_=wu_r[:, k, :])
            nc.sync.dma_start(out=wout_sb[:, k, :], in_=wout_r[:, k, :])
        nc.sync.dma_start(out=ones_k_sb[:], in_=ones_k[:])
        nc.sync.dma_start(out=ones_m_sb[:], in_=ones_m[:])

        # ---- phase 1: QKV projection + RoPE + transposes -------------
        # PSUM pools for phases 1 and 2 coexist within the 8 banks so the
        # scheduler can overlap projection with early attention blocks.
        with tc.tile_pool(name="p1psum", bufs=1, space="PSUM") as pp, \
             tc.tile_pool(name="p1sb", bufs=3) as sb:
            for rt in range(KC):
                # Q at [0:384] (bank0) and K at [512:896] (bank1) of one
                # 2-bank tile; V time-shares the same slot tag so phase-1
                # PSUM stays at 2 banks and phase-2 pools fit alongside.
                pqk = pp.tile([128, 1024], F32, tag="pqk")
                for k in range(6):
                    lhs = xT_sb[:, k, rt * 128:(rt + 1) * 128]
                    st, sp = (k == 0), (k == 5)
                    nc.tensor.matmul(pqk[:, 0:384], lhs, wqkv_sb[:, k, 0:384],
                                     start=st, stop=sp)
                    nc.tensor.matmul(pqk[:, 512:896], lhs,
                                     wqkv_sb[:, k, 384:768],
                                     start=st, stop=sp)
                rb, rt4 = rt // 4, rt % 4
                r0, r1 = rt * 128, (rt + 1) * 128
                c0, c1 = rt4 * 128, (rt4 + 1) * 128
                # Q/K to SBUF bf16 then RoPE in row layout
                q_t = sb.tile([128, NH, D], BF16, tag="qrow")
                k_t = sb.tile([128, NH, D], BF16, tag="krow")
                nc.scalar.copy(q_t[:], pqk[:, 0:384])
                nc.scalar.copy(k_t[:], pqk[:, 512:896])
                pv = pp.tile([128, 1024], F32, tag="pqk", name="pv")
                for k in range(6):
                    nc.tensor.matmul(pv[:, 0:384],
                                     xT_sb[:, k, rt * 128:(rt + 1) * 128],
                                     wqkv_sb[:, k, 768:1152],
                                     start=(k == 0), stop=(k == 5))
                # V straight to slab (bf16)
                nc.scalar.copy(v_sb[rb][:, rt4, :, :], pv[:, 0:384])
                cos_t = sb.tile([128, NH, D], BF16, tag="cos")
                sin_t = sb.tile([128, NH, D], BF16, tag="sin")
                nc.sync.dma_start(out=cos_t[:], in_=cosr[r0:r1, :])
                nc.sync.dma_start(out=sin_t[:], in_=sinr[r0:r1, :])
                for name, src in (("q", q_t), ("k", k_t)):
                    t1 = sb.tile([128, NH, D], BF16, tag="rope_t1")
                    t2 = sb.tile([128, NH, D], BF16, tag="rope_t2")
                    rr = sb.tile([128, NH, D], BF16, tag="rope_r")
                    nc.vector.tensor_mul(t1[:, :, 0:32], src[:, :, 32:64],
                                         sin_t[:, :, 0:32])
                    nc.vector.tensor_mul(t1[:, :, 32:64], src[:, :, 0:32],
                                         sin_t[:, :, 32:64])
                    nc.vector.tensor_mul(t2[:], src[:], cos_t[:])
                    nc.vector.tensor_add(rr[:], t1[:], t2[:])
                    dst = qt_sb if name == "q" else kt_sb
                    for p in range(NPAIR):
                        tp = pp.tile([128, 128], BF16, tag="tp", name="tp")
                        nc.tensor.transpose(tp[:], rr[:, 2 * p:2 * p + 2, :],
                                            ident_sb[:])
                        nc.scalar.copy(dst[rb][:, p, c0:c1], tp[:])

            # phase 1b: U^T projection (W stationary); silu applied here
            # (it does not depend on the AllGather) so the epilogue only
            # multiplies.
            for ct in range(CT):
                for rb in range(NRB):
                    pu = pp.tile([128, RB], F32, tag="pqk", name="pu")
                    for k in range(6):
                        nc.tensor.matmul(
                            pu[:], wu_sb[:, k, ct * 128:(ct + 1) * 128],
                            xT_sb[:, k, rb * RB:(rb + 1) * RB],
                            start=(k == 0), stop=(k == 5))
                    usig = sb.tile([128, RB], BF16, tag="usig")
                    nc.scalar.activation(usig[:], pu[:], AF.Sigmoid)
                    nc.vector.tensor_mul(
                        ut_sb[:, ct, rb * RB:(rb + 1) * RB], usig[:], pu[:])

            # ---- phase 2: attention + chunked AllGather --------------
            dram = es.enter_context(tc.tile_pool(name="agdram", bufs=4,
                                                 space="DRAM"))
            ag_outs = []
            p2 = ExitStack()
            scp = p2.enter_context(
                tc.tile_pool(name="p2sc", bufs=2, space="PSUM"))
            avp = p2.enter_context(
                tc.tile_pool(name="p2av", bufs=1, space="PSUM"))
            atp = p2.enter_context(tc.tile_pool(name="p2sb", bufs=17))
            for qb in range(NRB):
                nkc = 4 * qb + 4
                q0, q1 = qb * RB, (qb + 1) * RB
                for p in range(NPAIR):
                    av = avp.tile([128, RB], F32, tag="av")
                    # all scores matmuls of the head pair back-to-back so
                    # the in-order PE never stalls on a sigmoid (keeps the
                    # HAM clock gate open); sigmoids pipeline on ScalarE.
                    # one [128,1024] PSUM tile per key chunk holds BOTH
                    # heads' scores: the two matmuls target PE row groups
                    # 0-63 / 64-127 so each LDWEIGHTS overlaps the other
                    # head's matmul, and one big sigmoid serves both.
                    ats = {}
                    for kc in range(nkc):
                        sc = scp.tile([128, 1024], F32, tag="sc")
                        at = atp.tile([128, 1024], BF16, tag="at")
                        kslc = kt_sb[kc // 4]
                        for h01 in range(2):
                            b0 = 64 * h01
                            nc.tensor.matmul(
                                sc[:, h01 * RB:(h01 + 1) * RB],
                                kslc[b0:b0 + 64, p,
                                     (kc % 4) * 128:(kc % 4 + 1) * 128],
                                qt_sb[qb][b0:b0 + 64, p, :],
                                start=True, stop=True)
                        nc.scalar.activation(at[:], sc[:], AF.Sigmoid,
                                             scale=0.125)
                        t = kc - 4 * qb
                        if t >= 0:
                            for h01 in range(2):
                                nc.vector.tensor_mul(
                                    at[:, h01 * RB:(h01 + 1) * RB],
                                    at[:, h01 * RB:(h01 + 1) * RB],
                                    maskb_sb[:, 384 - 128 * t:896 - 128 * t])
                        ats[kc] = at
                    # A@V: the h0/h1 matmuls write disjoint PSUM partition
                    # halves (PE column groups 0-63 / 64-127) back-to-back,
                    # which the PE executes concurrently.
                    for kc in range(nkc):
                        for h01 in range(2):
                            b0 = 64 * h01
                            nc.tensor.matmul(
                                av[b0:b0 + 64, :],
                                v_sb[kc // 4][:, kc % 4, 2 * p + h01, :],
                                ats[kc][:, h01 * RB:(h01 + 1) * RB],
                                start=(kc == 0), stop=(kc == nkc - 1),
                                skip_group_check=True)
                    nc.vector.tensor_copy(ao_sb[qb][:, p, :], av[:])
                ag_in = dram.tile([NPAIR, 128, RB], BF16, tag="agin")
                ag_out = dram.tile([2, NPAIR, 128, RB], BF16, tag="agout")
                nc.gpsimd.dma_start(out=ag_in.rearrange("p i j -> i p j"),
                                    in_=ao_sb[qb][:])
                nc.gpsimd.collective_compute(
                    "AllGather", mybir.AluOpType.bypass,
                    replica_groups=pairs,
                    ins=[ag_in.opt()], outs=[ag_out.opt()])
                ag_outs.append(ag_out)
            p2.close()

        # ---- phase 3: LayerNorm + gate + out proj + residual ---------
        with tc.tile_pool(name="p3st", bufs=2, space="PSUM") as stp, \
             tc.tile_pool(name="p3bc", bufs=1, space="PSUM") as bcp, \
             tc.tile_pool(name="p3o", bufs=2, space="PSUM") as op, \
             tc.tile_pool(name="p3sb", bufs=2) as sb, \
             tc.tile_pool(name="p3small", bufs=2) as ssb:
            for rb in range(NRB):
                aot = sb.tile([128, 2, NPAIR, RB], BF16, tag="aot")
                nc.sync.dma_start(
                    out=aot[:],
                    in_=ag_outs[rb].rearrange("r p i j -> i r p j"))
                aotf = aot.rearrange("i r p j -> i (r p) j")   # [128, 6, RB]
                ssum = stp.tile([1, RB], F32, tag="ssum")
                qsum = stp.tile([1, RB], F32, tag="qsum")
                for ct in range(CT):
                    sq = sb.tile([128, RB], BF16, tag="sq")
                    nc.vector.tensor_mul(sq[:], aotf[:, ct, :], aotf[:, ct, :])
                    nc.tensor.matmul(ssum[:], ones_k_sb[:], aotf[:, ct, :],
                                     start=(ct == 0), stop=(ct == 5))
                    nc.tensor.matmul(qsum[:], ones_k_sb[:], sq[:],
                                     start=(ct == 0), stop=(ct == 5))
                mu = ssb.tile([1, RB], F32, tag="mu")
                musq = ssb.tile([1, RB], F32, tag="musq")
                var = ssb.tile([1, RB], F32, tag="var")
                std = ssb.tile([1, RB], F32, tag="musq", name="std")
                rstd = ssb.tile([1, RB], F32, tag="var", name="rstd")
                mu_b = ssb.tile([1, RB], BF16, tag="mub")
                rstd_b = ssb.tile([1, RB], BF16, tag="rstdb")
                nc.vector.tensor_scalar_mul(mu[:], ssum[:], 1.0 / HID)
                nc.vector.tensor_mul(musq[:], mu[:], mu[:])
                nc.vector.scalar_tensor_tensor(
                    var[:], qsum[:], 1.0 / HID, musq[:],
                    op0=mybir.AluOpType.mult, op1=mybir.AluOpType.subtract)
                eps_t = ssb.tile([1, 1], F32, tag="eps")
                nc.gpsimd.memset(eps_t[:], LN_EPS)
                nc.scalar.activation(std[:], var[:], AF.Sqrt, bias=eps_t[:])
                nc.vector.reciprocal_approx_fast(rstd[:], std[:])
                nc.vector.tensor_copy(mu_b[:], mu[:])
                nc.vector.tensor_copy(rstd_b[:], rstd[:])
                mu128 = bcp.tile([128, RB], F32, tag="mu128")
                rstd128 = bcp.tile([128, RB], F32, tag="rstd128")
                nc.tensor.matmul(mu128[:], ones_m_sb[:], mu_b[:],
                                 start=True, stop=True)
                nc.tensor.matmul(rstd128[:], ones_m_sb[:], rstd_b[:],
                                 start=True, stop=True)
                # stage broadcast stats to SBUF bf16 so the DVE apply chain
                # runs in its 2x bf16 mode (PSUM operands force 1x)
                mu_s = sb.tile([128, RB], BF16, tag="mus")
                rs_s = sb.tile([128, RB], BF16, tag="rss")
                nc.scalar.copy(mu_s[:], mu128[:])
                nc.scalar.copy(rs_s[:], rstd128[:])
                gated = sb.tile([128, CT, RB], BF16, tag="gated", bufs=1)
                for ct in range(CT):
                    d1 = sb.tile([128, RB], BF16, tag="d1")
                    d2 = sb.tile([128, RB], BF16, tag="d2")
                    nc.vector.tensor_sub(d1[:], aotf[:, ct, :], mu_s[:])
                    nc.vector.tensor_mul(d2[:], d1[:], rs_s[:])
                    nc.vector.tensor_mul(gated[:, ct, :], d2[:],
                                         ut_sb[:, ct, rb * RB:(rb + 1) * RB])
                for ctp in range(NPAIR):
                    po = op.tile([128, RB], F32, tag="po")
                    for ct in range(CT):
                        nc.tensor.matmul(
                            po[:], wout_sb[:, ct, ctp * 128:(ctp + 1) * 128],
                            gated[:, ct, :], start=(ct == 0), stop=(ct == 5))
                    rt_t = sb.tile([128, RB], F32, tag="resid")
                    nc.sync.dma_start(
                        out=rt_t[:],
                        in_=residT_r[:, ctp, rb * RB:(rb + 1) * RB])
                    o_t = sb.tile([128, RB], F32, tag="osb")
                    nc.vector.tensor_add(o_t[:], po[:], rt_t[:])
                    nc.gpsimd.dma_start(
                        out=out_r[:, ctp, rb * RB:(rb + 1) * RB], in_=o_t[:])


# ---------------------------------------------------------------------------
# host side
# ---------------------------------------------------------------------------

def prep_inputs(x, attn_mask, W_proj, b_proj, ln_gamma, ln_beta, W_out, b_out):
    x = np.asarray(x, dtype=np.float32)
    W_proj = np.asarray(W_proj, dtype=np.float32)
    b_proj = np.asarray(b_proj, dtype=np.float32)
    ln_gamma = np.asarray(ln_gamma, dtype=np.float32)
    ln_beta = np.asarray(ln_beta, dtype=np.float32)
    W_out = np.asarray(W_out, dtype=np.float32)
    b_out = np.asarray(b_out, dtype=np.float32)

    tril = np.tril(np.ones((S, S), dtype=bool))
    am = np.asarray(attn_mask)
    if not all(np.array_equal(am[b], tril) for b in range(am.shape[0])):
        raise ValueError("kernel specialized for causal attn_mask")
    if np.any(b_proj != 0) or np.any(ln_beta != 0):
        raise ValueError("kernel specialized for zero b_proj / ln_beta")

    bf = ml_dtypes.bfloat16
    cos, sin = _rope_tables()
    sinN = sin.copy()
    sinN[:, 0:32] = -sinN[:, 0:32]
    cosr = np.tile(cos, (1, NH)).astype(bf)
    sinr = np.tile(sinN, (1, NH)).astype(bf)

    iw = np.arange(896)[None, :]
    ii = np.arange(128)[:, None]
    maskb = (iw >= ii + 384).astype(bf)
    ident = np.eye(128, dtype=bf)
    ones_k = np.ones((128, 1), dtype=bf)
    ones_m = np.ones((1, 128), dtype=bf)

    Wg = (ln_gamma[:, None] * W_out).astype(np.float32)   # gamma folded
    U_c, V_c, Q_c, K_c = 0, HID, 2 * HID, 3 * HID

    in_maps = []
    for c in range(N_CORES):
        b, hh = c // 2, c % 2
        heads = range(NH * hh, NH * hh + NH)
        qcols = np.concatenate(
            [np.arange(Q_c + h * D, Q_c + (h + 1) * D) for h in heads])
        kcols = qcols - Q_c + K_c
        vcols = qcols - Q_c + V_c
        w_qkv = np.concatenate(
            [W_proj[:, qcols], W_proj[:, kcols], W_proj[:, vcols]],
            axis=1).astype(bf)
        w_u = W_proj[:, U_c:U_c + HID].astype(bf)
        w_out_half = Wg[:, hh * 384:(hh + 1) * 384].astype(bf)
        xTb = x[b].T                                       # [768, 2048]
        residT = (xTb[hh * 384:(hh + 1) * 384, :]
                  + b_out[hh * 384:(hh + 1) * 384, None]).astype(np.float32)
        in_maps.append(dict(
            xT=np.ascontiguousarray(xTb).astype(bf),
            w_qkv=np.ascontiguousarray(w_qkv),
            w_u=np.ascontiguousarray(w_u),
            w_out=np.ascontiguousarray(w_out_half),
            cosr=cosr, sinr=sinr, maskb=maskb, ident=ident,
            ones_k=ones_k, ones_m=ones_m,
            residT=np.ascontiguousarray(residT),
        ))
    return in_maps


def assemble(results, B=4):
    full = np.empty((B, S, HID), dtype=np.float32)
    for c in range(N_CORES):
        b, hh = c // 2, c % 2
        full[b, :, hh * 384:(hh + 1) * 384] = results[c]["out"].T
    return full


_NC_CACHE = {}


def get_nc(ndev=N_CORES):
    if ndev not in _NC_CACHE:
        pairs = [[i, i + 1] for i in range(0, ndev, 2)]
        _NC_CACHE[ndev] = build_nc(ndev, pairs)
    return _NC_CACHE[ndev]


def kernel(**inputs):
    in_maps = prep_inputs(**inputs)
    nc = get_nc(N_CORES)
    res = bass_utils.run_bass_kernel_spmd(
        nc, in_maps, core_ids=list(range(N_CORES)))
    return assemble(res.results)


# revision 24
# speedup vs baseline: 1.0573x; 1.0212x over previous
"""Sparse (sigmoid) attention block on 8 TRN2 NeuronCores.

Sharding: core c = (batch b=c//2, head-half hh=c%2).  Each core computes the
QKV projection + RoPE + causal sigmoid-attention for 6 of the 12 heads over
the full 2048-row sequence of its batch (graphs are identical across cores;
only the weight/input shards differ).  A 2-rank AllGather inside each core
pair exchanges the per-head attention outputs so both cores see the full
hidden dim for LayerNorm; the epilogue output projection is column-split
across the pair.  All heavy compute in bf16 with f32 PSUM accumulation.

Layouts: projections produce row-major [rows, cols] tiles; RoPE is applied
in row layout (free-axis half-rotation via a sign-folded sin table); PE
transposes produce head-pair-stacked Q^T/K^T [128, pair, seq]; attention
output, LayerNorm, gating and the output projection all live in transposed
[hidden, rows] layout so no further transposes are needed; the host
transposes the final [384, 2048] per-core outputs back.
"""

import numpy as np
import ml_dtypes

import concourse.bass as bass
import concourse.bacc as bacc
import concourse.mybir as mybir
import concourse.tile as tile
from concourse import bass_utils

BF16 = mybir.dt.bfloat16
F32 = mybir.dt.float32
AF = mybir.ActivationFunctionType

S = 2048          # sequence length
HID = 768         # hidden
NHEADS = 12       # total heads
D = 64            # head dim
NH = 6            # heads per core
NPAIR = 3         # head pairs per core
KC = 16           # key chunks of 128
NRB = 4           # row blocks of 512
RB = 512
CT = 6            # hidden c-tiles of 128
LN_EPS = 1e-8
N_CORES = 8


def _rope_tables():
    inv_freq = 1.0 / (10000.0 ** (np.arange(0, D, 2, dtype=np.float64) / D))
    t = np.arange(S, dtype=np.float64)
    freqs = np.outer(t, inv_freq)                      # [S, 32]
    emb = np.concatenate([freqs, freqs], axis=-1)      # [S, 64]
    return np.cos(emb).astype(np.float32), np.sin(emb).astype(np.float32)


def build_nc(ndev, pairs):
    """Emit the per-core Bass/Tile graph (identical for every core)."""
    nc = bacc.Bacc("TRN2", target_bir_lowering=False, debug=False,
                   num_devices=ndev)

    def din(name, shape, dt):
        return nc.dram_tensor(name, shape, dt, kind="ExternalInput").ap()

    xT = din("xT", [HID, S], BF16)
    w_qkv = din("w_qkv", [HID, 3 * NH * D], BF16)      # [768, 1152] Q|K|V
    w_u = din("w_u", [HID, HID], BF16)
    w_out = din("w_out", [HID, 384], BF16)             # gamma-folded half
    cosr = din("cosr", [S, NH * D], BF16)
    sinr = din("sinr", [S, NH * D], BF16)              # sign-folded sin
    maskb = din("maskb", [128, 896], BF16)
    ident = din("ident", [128, 128], BF16)
    ones_k = din("ones_k", [128, 1], BF16)
    ones_m = din("ones_m", [1, 128], BF16)
    residT = din("residT", [384, S], F32)              # x^T half + b_out
    out = nc.dram_tensor("out", [384, S], F32, kind="ExternalOutput").ap()

    xT_r = xT.rearrange("(k p) s -> p k s", p=128)         # [128, 6, S]
    wqkv_r = w_qkv.rearrange("(k p) c -> p k c", p=128)    # [128, 6, 1152]
    wu_r = w_u.rearrange("(k p) c -> p k c", p=128)
    wout_r = w_out.rearrange("(k p) c -> p k c", p=128)    # [128, 6, 384]
    residT_r = residT.rearrange("(c p) s -> p c s", p=128)  # [128, 3, S]
    out_r = out.rearrange("(c p) s -> p c s", p=128)

    with tile.TileContext(nc) as tc:
        _emit(nc, tc, pairs, xT_r, wqkv_r, wu_r, wout_r, cosr, sinr,
              maskb, ident, ones_k, ones_m, residT_r, out_r)
    nc.compile()
    return nc


def _emit(nc, tc, pairs, xT_r, wqkv_r, wu_r, wout_r, cosr, sinr,
          maskb, ident, ones_k, ones_m, residT_r, out_r):
    from contextlib import ExitStack
    es = ExitStack()
    with es:
        # ---- resident SBUF tensors -----------------------------------
        res = es.enter_context(tc.tile_pool(name="resident", bufs=1))
        xT_sb = res.tile([128, 6, S], BF16, tag="xT")
        wqkv_sb = res.tile([128, 6, 3 * NH * D], BF16, tag="wqkv")
        wu_sb = res.tile([128, 6, HID], BF16, tag="wu")
        wout_sb = res.tile([128, 6, 384], BF16, tag="wout")
        maskb_sb = res.tile([128, 896], BF16, tag="maskb")
        ident_sb = res.tile([128, 128], BF16, tag="ident")
        ones_k_sb = res.tile([128, 1], BF16, tag="onesk")
        ones_m_sb = res.tile([1, 128], BF16, tag="onesm")
        # per-row-block slabs so attention on early query blocks can start
        # while projection of later rows is still running
        qt_sb = [res.tile([128, NPAIR, RB], BF16, tag=f"qt{i}", name=f"qt{i}")
                 for i in range(NRB)]                          # Q^T slabs
        kt_sb = [res.tile([128, NPAIR, RB], BF16, tag=f"kt{i}", name=f"kt{i}")
                 for i in range(NRB)]                          # K^T slabs
        v_sb = [res.tile([128, 4, NH, D], BF16, tag=f"v{i}", name=f"v{i}")
                for i in range(NRB)]                           # V slabs
        ao_sb = [res.tile([128, NPAIR, RB], BF16, tag=f"ao{i}", name=f"ao{i}")
                 for i in range(NRB)]                          # attn out^T
        ut_sb = res.tile([128, CT, S], BF16, tag="ut")         # U^T slab

        # projection-critical loads first so the first matmuls start early
        for k in range(6):
            nc.sync.dma_start(out=xT_sb[:, k, :], in_=xT_r[:, k, :])
            nc.sync.dma_start(out=wqkv_sb[:, k, :], in_=wqkv_r[:, k, :])
        nc.sync.dma_start(out=ident_sb[:], in_=ident[:])
        nc.sync.dma_start(out=maskb_sb[:], in_=maskb[:])
        for k in range(6):
            nc.sync.dma_start(out=wu_sb[:, k, :], in_=wu_r[:, k, :])
            nc.sync.dma_start(out=wout_sb[:, k, :], in_=wout_r[:, k, :])
        nc.sync.dma_start(out=ones_k_sb[:], in_=ones_k[:])
        nc.sync.dma_start(out=ones_m_sb[:], in_=ones_m[:])

        # ---- phase 1: QKV projection + RoPE + transposes -------------
        # PSUM pools for phases 1 and 2 coexist within the 8 banks so the
        # scheduler can overlap projection with early attention blocks.
        with tc.tile_pool(name="p1psum", bufs=1, space="PSUM") as pp, \
             tc.tile_pool(name="p1sb", bufs=3) as sb:
            for rt in range(KC):
                # Q at [0:384] (bank0) and K at [512:896] (bank1) of one
                # 2-bank tile; V time-shares the same slot tag so phase-1
                # PSUM stays at 2 banks and phase-2 pools fit alongside.
                pqk = pp.tile([128, 1024], F32, tag="pqk")
                for k in range(6):
                    lhs = xT_sb[:, k, rt * 128:(rt + 1) * 128]
                    st, sp = (k == 0), (k == 5)
                    nc.tensor.matmul(pqk[:, 0:384], lhs, wqkv_sb[:, k, 0:384],
                                     start=st, stop=sp)
                    nc.tensor.matmul(pqk[:, 512:896], lhs,
                                     wqkv_sb[:, k, 384:768],
                                     start=st, stop=sp)
                rb, rt4 = rt // 4, rt % 4
                r0, r1 = rt * 128, (rt + 1) * 128
                c0, c1 = rt4 * 128, (rt4 + 1) * 128
                # Q/K to SBUF bf16 then RoPE in row layout
                q_t = sb.tile([128, NH, D], BF16, tag="qrow")
                k_t = sb.tile([128, NH, D], BF16, tag="krow")
                nc.scalar.copy(q_t[:], pqk[:, 0:384])
                nc.scalar.copy(k_t[:], pqk[:, 512:896])
                pv = pp.tile([128, 1024], F32, tag="pqk", name="pv")
                for k in range(6):
                    nc.tensor.matmul(pv[:, 0:384],
                                     xT_sb[:, k, rt * 128:(rt + 1) * 128],
                                     wqkv_sb[:, k, 768:1152],
                                     start=(k == 0), stop=(k == 5))
                # V straight to slab (bf16)
                nc.scalar.copy(v_sb[rb][:, rt4, :, :], pv[:, 0:384])
                cos_t = sb.tile([128, NH, D], BF16, tag="cos")
                sin_t = sb.tile([128, NH, D], BF16, tag="sin")
                nc.sync.dma_start(out=cos_t[:], in_=cosr[r0:r1, :])
                nc.sync.dma_start(out=sin_t[:], in_=sinr[r0:r1, :])
                for name, src in (("q", q_t), ("k", k_t)):
                    t1 = sb.tile([128, NH, D], BF16, tag="rope_t1")
                    t2 = sb.tile([128, NH, D], BF16, tag="rope_t2")
                    rr = sb.tile([128, NH, D], BF16, tag="rope_r")
                    nc.vector.tensor_mul(t1[:, :, 0:32], src[:, :, 32:64],
                                         sin_t[:, :, 0:32])
                    nc.vector.tensor_mul(t1[:, :, 32:64], src[:, :, 0:32],
                                         sin_t[:, :, 32:64])
                    nc.vector.tensor_mul(t2[:], src[:], cos_t[:])
                    nc.vector.tensor_add(rr[:], t1[:], t2[:])
                    dst = qt_sb if name == "q" else kt_sb
                    for p in range(NPAIR):
                        tp = pp.tile([128, 128], BF16, tag="tp", name="tp")
                        nc.tensor.transpose(tp[:], rr[:, 2 * p:2 * p + 2, :],
                                            ident_sb[:])
                        nc.vector.tensor_copy(dst[rb][:, p, c0:c1], tp[:])

            # phase 1b: U^T projection (W stationary); silu applied here
            # (it does not depend on the AllGather) so the epilogue only
            # multiplies.
            for ct in range(CT):
                for rb in range(NRB):
                    pu = pp.tile([128, RB], F32, tag="pqk", name="pu")
                    for k in range(6):
                        nc.tensor.matmul(
                            pu[:], wu_sb[:, k, ct * 128:(ct + 1) * 128],
                            xT_sb[:, k, rb * RB:(rb + 1) * RB],
                            start=(k == 0), stop=(k == 5))
                    usig = sb.tile([128, RB], BF16, tag="usig")
                    nc.scalar.activation(usig[:], pu[:], AF.Sigmoid)
                    nc.vector.tensor_mul(
                        ut_sb[:, ct, rb * RB:(rb + 1) * RB], usig[:], pu[:])

            # ---- phase 2: attention + chunked AllGather --------------
            dram = es.enter_context(tc.tile_pool(name="agdram", bufs=4,
                                                 space="DRAM"))
            ag_outs = []
            p2 = ExitStack()
            scp = p2.enter_context(
                tc.tile_pool(name="p2sc", bufs=2, space="PSUM"))
            avp = p2.enter_context(
                tc.tile_pool(name="p2av", bufs=1, space="PSUM"))
            atp = p2.enter_context(tc.tile_pool(name="p2sb", bufs=17))
            for qb in range(NRB):
                nkc = 4 * qb + 4
                q0, q1 = qb * RB, (qb + 1) * RB
                for p in range(NPAIR):
                    av = avp.tile([128, RB], F32, tag="av")
                    # all scores matmuls of the head pair back-to-back so
                    # the in-order PE never stalls on a sigmoid (keeps the
                    # HAM clock gate open); sigmoids pipeline on ScalarE.
                    # one [128,1024] PSUM tile per key chunk holds BOTH
                    # heads' scores: the two matmuls target PE row groups
                    # 0-63 / 64-127 so each LDWEIGHTS overlaps the other
                    # head's matmul, and one big sigmoid serves both.
                    ats = {}
                    for kc in range(nkc):
                        sc = scp.tile([128, 1024], F32, tag="sc")
                        at = atp.tile([128, 1024], BF16, tag="at")
                        kslc = kt_sb[kc // 4]
                        for h01 in range(2):
                            b0 = 64 * h01
                            nc.tensor.matmul(
                                sc[:, h01 * RB:(h01 + 1) * RB],
                                kslc[b0:b0 + 64, p,
                                     (kc % 4) * 128:(kc % 4 + 1) * 128],
                                qt_sb[qb][b0:b0 + 64, p, :],
                                start=True, stop=True)
                        nc.scalar.activation(at[:], sc[:], AF.Sigmoid,
                                             scale=0.125)
                        t = kc - 4 * qb
                        if t >= 0:
                            for h01 in range(2):
                                nc.vector.tensor_mul(
                                    at[:, h01 * RB:(h01 + 1) * RB],
                                    at[:, h01 * RB:(h01 + 1) * RB],
                                    maskb_sb[:, 384 - 128 * t:896 - 128 * t])
                        ats[kc] = at
                    # A@V: the h0/h1 matmuls write disjoint PSUM partition
                    # halves (PE column groups 0-63 / 64-127) back-to-back,
                    # which the PE executes concurrently.
                    for kc in range(nkc):
                        for h01 in range(2):
                            b0 = 64 * h01
                            nc.tensor.matmul(
                                av[b0:b0 + 64, :],
                                v_sb[kc // 4][:, kc % 4, 2 * p + h01, :],
                                ats[kc][:, h01 * RB:(h01 + 1) * RB],
                                start=(kc == 0), stop=(kc == nkc - 1),
                                skip_group_check=True)
                    nc.vector.tensor_copy(ao_sb[qb][:, p, :], av[:])
                ag_in = dram.tile([NPAIR, 128, RB], BF16, tag="agin")
                ag_out = dram.tile([2, NPAIR, 128, RB], BF16, tag="agout")
                nc.gpsimd.dma_start(out=ag_in.rearrange("p i j -> i p j"),
                                    in_=ao_sb[qb][:])
                nc.gpsimd.collective_compute(
                    "AllGather", mybir.AluOpType.bypass,
                    replica_groups=pairs,
                    ins=[ag_in.opt()], outs=[ag_out.opt()])
                ag_outs.append(ag_out)
            p2.close()

        # ---- phase 3: LayerNorm + gate + out proj + residual ---------
        with tc.tile_pool(name="p3st", bufs=2, space="PSUM") as stp, \
             tc.tile_pool(name="p3bc", bufs=1, space="PSUM") as bcp, \
             tc.tile_pool(name="p3o", bufs=2, space="PSUM") as op, \
             tc.tile_pool(name="p3sb", bufs=2) as sb, \
             tc.tile_pool(name="p3small", bufs=2) as ssb:
            for rb in range(NRB):
                aot = sb.tile([128, 2, NPAIR, RB], BF16, tag="aot")
                nc.sync.dma_start(
                    out=aot[:],
                    in_=ag_outs[rb].rearrange("r p i j -> i r p j"))
                aotf = aot.rearrange("i r p j -> i (r p) j")   # [128, 6, RB]
                ssum = stp.tile([1, RB], F32, tag="ssum")
                qsum = stp.tile([1, RB], F32, tag="qsum")
                for ct in range(CT):
                    sq = sb.tile([128, RB], BF16, tag="sq")
                    nc.vector.tensor_mul(sq[:], aotf[:, ct, :], aotf[:, ct, :])
                    nc.tensor.matmul(ssum[:], ones_k_sb[:], aotf[:, ct, :],
                                     start=(ct == 0), stop=(ct == 5))
                    nc.tensor.matmul(qsum[:], ones_k_sb[:], sq[:],
                                     start=(ct == 0), stop=(ct == 5))
                mu = ssb.tile([1, RB], F32, tag="mu")
                musq = ssb.tile([1, RB], F32, tag="musq")
                var = ssb.tile([1, RB], F32, tag="var")
                std = ssb.tile([1, RB], F32, tag="musq", name="std")
                rstd = ssb.tile([1, RB], F32, tag="var", name="rstd")
                mu_b = ssb.tile([1, RB], BF16, tag="mub")
                rstd_b = ssb.tile([1, RB], BF16, tag="rstdb")
                nc.vector.tensor_scalar_mul(mu[:], ssum[:], 1.0 / HID)
                nc.vector.tensor_mul(musq[:], mu[:], mu[:])
                nc.vector.scalar_tensor_tensor(
                    var[:], qsum[:], 1.0 / HID, musq[:],
                    op0=mybir.AluOpType.mult, op1=mybir.AluOpType.subtract)
                eps_t = ssb.tile([1, 1], F32, tag="eps")
                nc.gpsimd.memset(eps_t[:], LN_EPS)
                nc.scalar.activation(std[:], var[:], AF.Sqrt, bias=eps_t[:])
                nc.vector.reciprocal_approx_fast(rstd[:], std[:])
                nc.vector.tensor_copy(mu_b[:], mu[:])
                nc.vector.tensor_copy(rstd_b[:], rstd[:])
                mu128 = bcp.tile([128, RB], F32, tag="mu128")
                rstd128 = bcp.tile([128, RB], F32, tag="rstd128")
                nc.tensor.matmul(mu128[:], ones_m_sb[:], mu_b[:],
                                 start=True, stop=True)
                nc.tensor.matmul(rstd128[:], ones_m_sb[:], rstd_b[:],
                                 start=True, stop=True)
                # stage broadcast stats to SBUF bf16 so the DVE apply chain
                # runs in its 2x bf16 mode (PSUM operands force 1x)
                mu_s = sb.tile([128, RB], BF16, tag="mus")
                rs_s = sb.tile([128, RB], BF16, tag="rss")
                nc.scalar.copy(mu_s[:], mu128[:])
                nc.scalar.copy(rs_s[:], rstd128[:])
                gated = sb.tile([128, CT, RB], BF16, tag="gated", bufs=1)
                for ct in range(CT):
                    d1 = sb.tile([128, RB], BF16, tag="d1")
                    d2 = sb.tile([128, RB], BF16, tag="d2")
                    nc.vector.tensor_sub(d1[:], aotf[:, ct, :], mu_s[:])
                    nc.vector.tensor_mul(d2[:], d1[:], rs_s[:])
                    nc.vector.tensor_mul(gated[:, ct, :], d2[:],
                                         ut_sb[:, ct, rb * RB:(rb + 1) * RB])
                for ctp in range(NPAIR):
                    po = op.tile([128, RB], F32, tag="po")
                    for ct in range(CT):
                        nc.tensor.matmul(
                            po[:], wout_sb[:, ct, ctp * 128:(ctp + 1) * 128],
                            gated[:, ct, :], start=(ct == 0), stop=(ct == 5))
                    rt_t = sb.tile([128, RB], F32, tag="resid")
                    nc.sync.dma_start(
                        out=rt_t[:],
                        in_=residT_r[:, ctp, rb * RB:(rb + 1) * RB])
                    o_t = sb.tile([128, RB], F32, tag="osb")
                    nc.vector.tensor_add(o_t[:], po[:], rt_t[:])
                    nc.gpsimd.dma_start(
                        out=out_r[:, ctp, rb * RB:(rb + 1) * RB], in_=o_t[:])


# ---------------------------------------------------------------------------
# host side
# ---------------------------------------------------------------------------

def prep_inputs(x, attn_mask, W_proj, b_proj, ln_gamma, ln_beta, W_out, b_out):
    x = np.asarray(x, dtype=np.float32)
    W_proj = np.asarray(W_proj, dtype=np.float32)
    b_proj = np.asarray(b_proj, dtype=np.float32)
    ln_gamma = np.asarray(ln_gamma, dtype=np.float32)
    ln_beta = np.asarray(ln_beta, dtype=np.float32)
    W_out = np.asarray(W_out, dtype=np.float32)
    b_out = np.asarray(b_out, dtype=np.float32)

    tril = np.tril(np.ones((S, S), dtype=bool))
    am = np.asarray(attn_mask)
    if not all(np.array_equal(am[b], tril) for b in range(am.shape[0])):
        raise ValueError("kernel specialized for causal attn_mask")
    if np.any(b_proj != 0) or np.any(ln_beta != 0):
        raise ValueError("kernel specialized for zero b_proj / ln_beta")

    bf = ml_dtypes.bfloat16
    cos, sin = _rope_tables()
    sinN = sin.copy()
    sinN[:, 0:32] = -sinN[:, 0:32]
    cosr = np.tile(cos, (1, NH)).astype(bf)
    sinr = np.tile(sinN, (1, NH)).astype(bf)

    iw = np.arange(896)[None, :]
    ii = np.arange(128)[:, None]
    maskb = (iw >= ii + 384).astype(bf)
    ident = np.eye(128, dtype=bf)
    ones_k = np.ones((128, 1), dtype=bf)
    ones_m = np.ones((1, 128), dtype=bf)

    Wg = (ln_gamma[:, None] * W_out).astype(np.float32)   # gamma folded
    U_c, V_c, Q_c, K_c = 0, HID, 2 * HID, 3 * HID

    in_maps = []
    for c in range(N_CORES):
        b, hh = c // 2, c % 2
        heads = range(NH * hh, NH * hh + NH)
        qcols = np.concatenate(
            [np.arange(Q_c + h * D, Q_c + (h + 1) * D) for h in heads])
        kcols = qcols - Q_c + K_c
        vcols = qcols - Q_c + V_c
        w_qkv = np.concatenate(
            [W_proj[:, qcols], W_proj[:, kcols], W_proj[:, vcols]],
            axis=1).astype(bf)
        w_u = W_proj[:, U_c:U_c + HID].astype(bf)
        w_out_half = Wg[:, hh * 384:(hh + 1) * 384].astype(bf)
        xTb = x[b].T                                       # [768, 2048]
        residT = (xTb[hh * 384:(hh + 1) * 384, :]
                  + b_out[hh * 384:(hh + 1) * 384, None]).astype(np.float32)
        in_maps.append(dict(
            xT=np.ascontiguousarray(xTb).astype(bf),
            w_qkv=np.ascontiguousarray(w_qkv),
            w_u=np.ascontiguousarray(w_u),
            w_out=np.ascontiguousarray(w_out_half),
            cosr=cosr, sinr=sinr, maskb=maskb, ident=ident,
            ones_k=ones_k, ones_m=ones_m,
            residT=np.ascontiguousarray(residT),
        ))
    return in_maps


def assemble(results, B=4):
    full = np.empty((B, S, HID), dtype=np.float32)
    for c in range(N_CORES):
        b, hh = c // 2, c % 2
        full[b, :, hh * 384:(hh + 1) * 384] = results[c]["out"].T
    return full


_NC_CACHE = {}


def get_nc(ndev=N_CORES):
    if ndev not in _NC_CACHE:
        pairs = [[i, i + 1] for i in range(0, ndev, 2)]
        _NC_CACHE[ndev] = build_nc(ndev, pairs)
    return _NC_CACHE[ndev]


def kernel(**inputs):
    in_maps = prep_inputs(**inputs)
    nc = get_nc(N_CORES)
    res = bass_utils.run_bass_kernel_spmd(
        nc, in_maps, core_ids=list(range(N_CORES)))
    return assemble(res.results)
